# revision 5
# baseline (speedup 1.0000x reference)
"""MinGRU (2-layer) Trainium2 Bass kernel.

Problem: h[8,4096,1024] f32, W0/W1 [1024,3072] f32.
Per layer: z = h @ W; hidden,gate,proj = split(z);
  a = sigmoid(-gate); g_hidden = relu(hidden) + min(sigmoid(hidden), 0.5)
  scan: out_t = a_t*out_{t-1} + (1-a_t)*g_hidden_t   (DVE scan, fp32 state)
  h = sigmoid(proj)*out + (1-sigmoid(proj))*h

Sharding: batch row per core (B=8 over 8 cores), weights replicated.
On-core layout: [feat(partitions), time] throughout; host pre-transposes the
inputs and post-transposes the [H,T] fp16 output during unshard, so the PE
does zero transposes.

Matmul precision: contraction K=1024 is split — the first NK8*256 rows run
as fp8e4 DoubleRow matmuls (2 k-tiles per MM, half the PE issue slots), the
rest as fp16 matmuls, all accumulating into the same fp32 PSUM. Weights are
pre-scaled by WS=16 on host (keeps fp8 out of subnormals); ACT un-scales via
its activation scale input. Measured end-to-end rel L2 error ~1.3e-2 at
NK8=2 (vs the 2e-2 gate; fp16-only is ~5.5e-4).

Engine balance per [128,512] block (cost model): ACT 3 sigmoids (612ns ea),
Pool relu+2 fused stt (427ns ea), DVE scan (594ns) + 3 fp16 tensor_tensor
(327ns ea). All under the PE matmul stream, which is the bottleneck.
"""

import os
import sys

if "/opt/trn_rl_repo" not in sys.path:
    sys.path.insert(0, "/opt/trn_rl_repo")

from contextlib import ExitStack

import numpy as np

import concourse.bass as bass
import concourse.tile as tile
from concourse import bacc, mybir
from concourse import bass_utils

T, H, H3 = 4096, 1024, 3072
TC = 512                 # time chunk (= one PSUM bank of fp32)
NCHUNK = T // TC
NFB = H // 128           # feature blocks (partition tiles) = k-tiles total
NK8 = int(os.environ.get("MINGRU_NK8", "2"))  # fp8 DoubleRow k-tile PAIRS
NK16 = NFB - 2 * NK8     # fp16 k-tiles
K8 = NK8 * 256           # contraction rows done in fp8
WS = 16.0                # weight pre-scale; PSUM holds WS*z, ACT divides out
F32 = mybir.dt.float32
F16 = mybir.dt.float16
F8 = mybir.dt.float8e4
NP8 = mybir.dt.np(F8)
ACT = mybir.ActivationFunctionType
ALU = mybir.AluOpType
DRMODE = mybir.MatmulPerfMode.DoubleRow


def _emit_elemwise(nc, ew, carries, rhs16, h1T16, h1T8, y, ph, pg, pp, i, li, f):
    s_ = ew.tile([128, TC], F16, tag="s")
    nc.scalar.activation(s_[:], ph[:], ACT.Sigmoid, scale=1.0 / WS)
    a_ = ew.tile([128, TC], F16, tag="a")
    nc.scalar.activation(a_[:], pg[:], ACT.Sigmoid, scale=-1.0 / WS)
    g_ = ew.tile([128, TC], F16, tag="g")
    nc.scalar.activation(g_[:], pp[:], ACT.Sigmoid, scale=1.0 / WS)
    # GPSIMD cannot touch PSUM and only runs tensor_tensor/tensor_scalar/
    # copy, so: relu reads PSUM on DVE, the stt fusions are decomposed into
    # cheap DVE 4x-fp16 tensor_scalar ops + Pool tensor_tensor ops.
    rh = ew.tile([128, TC], F16, tag="rh")
    nc.vector.tensor_scalar(rh[:], ph[:], 0.0, 1.0 / WS, op0=ALU.max, op1=ALU.mult)
    smin = ew.tile([128, TC], F16, tag="smin")
    nc.vector.tensor_scalar_min(smin[:], s_[:], 0.5)
    am1 = ew.tile([128, TC], F16, tag="am1")
    nc.gpsimd.tensor_scalar_add(am1[:], a_[:], -1.0)
    gh = ew.tile([128, TC], F16, tag="gh")
    nc.gpsimd.tensor_tensor(gh[:], smin[:], rh[:], op=ALU.add)
    negb = ew.tile([128, TC], F16, tag="negb")
    nc.gpsimd.tensor_tensor(negb[:], am1[:], gh[:], op=ALU.mult)
    sc = ew.tile([128, TC], F16, tag="sc")
    col = li * NFB + f
    init = 0.0 if i == 0 else carries[:, col:col + 1]
    nc.vector.tensor_tensor_scan(sc[:], a_[:], negb[:], init,
                                 op0=ALU.mult, op1=ALU.subtract)
    if i < NCHUNK - 1:
        nc.vector.tensor_copy(carries[:, col:col + 1], sc[:, TC - 1:TC])
    # highway: h_out = h_in + g*(sc - h_in)
    d = ew.tile([128, TC], F16, tag="d")
    nc.vector.tensor_sub(d[:], sc[:], rhs16[:, f, :])
    m = ew.tile([128, TC], F16, tag="m")
    nc.vector.tensor_mul(m[:], g_[:], d[:])
    if li == 0:
        nc.gpsimd.tensor_add(h1T16[:, f, :], m[:], rhs16[:, f, :])
        if f < 2 * NK8:
            nc.gpsimd.tensor_copy(h1T8[:, f, :], h1T16[:, f, :])
    else:
        yt = ew.tile([128, TC], F16, tag="yt", bufs=3)
        nc.gpsimd.tensor_add(yt[:], m[:], rhs16[:, f, :])
        nc.sync.dma_start(y[f * 128:(f + 1) * 128, i * TC:(i + 1) * TC], yt[:])


def _emit_body(tc_, y, hT16d, hT8d, w16_sb, w8_sb, pools):
    nc = tc_.nc
    hT_pool, mm_psum, ew, carry_pool = pools
    carries = carry_pool.tile([128, 2 * NFB], F16)

    def emit_layer(i, li, rhs16, rhs8, h1T16, h1T8):
        for f in range(NFB):
            ph = mm_psum.tile([128, TC], F32, tag="ph")
            pg = mm_psum.tile([128, TC], F32, tag="pg")
            pp = mm_psum.tile([128, TC], F32, tag="pp")
            nstep = NK8 + NK16
            step = 0
            for kp in range(NK8):
                st = dict(start=(step == 0), stop=(step == nstep - 1))
                lw8 = w8_sb[li][kp]
                r8 = rhs8[:, 2 * kp:2 * kp + 2, :]
                nc.tensor.matmul(ph[:], lw8[:, :, f * 128:(f + 1) * 128],
                                 r8, perf_mode=DRMODE, **st)
                nc.tensor.matmul(pg[:], lw8[:, :, H + f * 128:H + (f + 1) * 128],
                                 r8, perf_mode=DRMODE, **st)
                nc.tensor.matmul(pp[:], lw8[:, :, 2 * H + f * 128:2 * H + (f + 1) * 128],
                                 r8, perf_mode=DRMODE, **st)
                step += 1
            for j in range(NK16):
                st = dict(start=(step == 0), stop=(step == nstep - 1))
                lw = w16_sb[li][j]
                r = rhs16[:, 2 * NK8 + j, :]
                nc.tensor.matmul(ph[:], lw[:, f * 128:(f + 1) * 128], r, **st)
                nc.tensor.matmul(pg[:], lw[:, H + f * 128:H + (f + 1) * 128], r, **st)
                nc.tensor.matmul(pp[:], lw[:, 2 * H + f * 128:2 * H + (f + 1) * 128],
                                 r, **st)
                step += 1
            _emit_elemwise(nc, ew, carries, rhs16, h1T16, h1T8, y,
                           ph, pg, pp, i, li, f)

    # Layer-2 runs one chunk behind layer-1 so the PE never stalls on the
    # DVE/Pool chain that produces layer-2's rhs.
    prev = None
    for i in range(NCHUNK):
        hT16 = hT_pool.tile([128, NFB, TC], F16, tag="hT")
        for k in range(NFB):
            nc.sync.dma_start(hT16[:, k, :],
                              hT16d[k * 128:(k + 1) * 128, i * TC:(i + 1) * TC])
        hT8 = None
        if NK8:
            hT8 = hT_pool.tile([128, 2 * NK8, TC], F8, tag="hT8")
            for k in range(2 * NK8):
                nc.sync.dma_start(hT8[:, k, :],
                                  hT8d[k * 128:(k + 1) * 128, i * TC:(i + 1) * TC])
        h1T16 = hT_pool.tile([128, NFB, TC], F16, tag="h1T")
        h1T8 = None
        if NK8:
            h1T8 = hT_pool.tile([128, 2 * NK8, TC], F8, tag="h1T8", name="h1T8")
        emit_layer(i, 0, hT16, hT8, h1T16, h1T8)
        if prev is not None:
            emit_layer(i - 1, 1, prev[0], prev[1], None, None)
        prev = (h1T16, h1T8)
    emit_layer(NCHUNK - 1, 1, prev[0], prev[1], None, None)


def build_nc(loop_iters: int = 1):
    """Build + compile the per-core Bass program (SPMD across 8 cores)."""
    nc = bacc.Bacc("TRN2", target_bir_lowering=False, debug=False,
                   enable_asserts=False, num_devices=8)
    hT16d = nc.dram_tensor("hT16", [H, T], F16, kind="ExternalInput").ap()
    hT8d = (nc.dram_tensor("hT8", [K8, T], F8, kind="ExternalInput").ap()
            if NK8 else None)
    w16d = nc.dram_tensor("w16", [2, H - K8, H3], F16, kind="ExternalInput").ap()
    w8d = (nc.dram_tensor("w8", [2, K8, H3], F8, kind="ExternalInput").ap()
           if NK8 else None)
    y = nc.dram_tensor("y", [H, T], F16, kind="ExternalOutput").ap()

    with tile.TileContext(nc) as tc_:
        with ExitStack() as ctx:
            wpool = ctx.enter_context(tc_.tile_pool(name="w", bufs=1))
            const = ctx.enter_context(tc_.tile_pool(name="const", bufs=1))
            hT_pool = ctx.enter_context(tc_.tile_pool(name="hT", bufs=2))
            mm_psum = ctx.enter_context(
                tc_.tile_pool(name="mmp", bufs=2, space="PSUM"))
            ew = ctx.enter_context(tc_.tile_pool(name="ew", bufs=2))
            carry_pool = ctx.enter_context(tc_.tile_pool(name="carry", bufs=1))

            # weights: layer-1 tiles first so chunk-0 matmuls gate on the
            # least possible DMA
            w16_sb = [[wpool.tile([128, H3], F16, name=f"w16_{li}_{j}",
                                  tag=f"w16_{li}_{j}")
                       for j in range(NK16)] for li in range(2)]
            w8_sb = [[wpool.tile([128, 2, H3], F8, name=f"w8_{li}_{kp}",
                                 tag=f"w8_{li}_{kp}")
                      for kp in range(NK8)] for li in range(2)]
            for li in range(2):
                for kp in range(NK8):
                    for jj in range(2):
                        nc.sync.dma_start(
                            w8_sb[li][kp][:, jj, :],
                            w8d[li, kp * 256 + jj * 128:kp * 256 + (jj + 1) * 128, :])
                for j in range(NK16):
                    nc.sync.dma_start(w16_sb[li][j][:],
                                      w16d[li, j * 128:(j + 1) * 128, :])

            # PE clock (HAM) warmup + ACT sigmoid-table preload, fed from a
            # memset tile so nothing gates on DMA.
            warm = const.tile([128, 128], F32, name="warm")
            nc.vector.memset(warm[:], 0.0)
            warm_ps = mm_psum.tile([128, TC], F32, tag="ph")
            for _ in range(16):
                nc.tensor.matmul(warm_ps[:, 0:128], warm[:], warm[:],
                                 start=True, stop=True)
            warm_sb = ew.tile([128, TC], F16, tag="s")
            nc.scalar.activation(warm_sb[:, 0:1], warm_ps[:, 0:1], ACT.Sigmoid)

            pools = (hT_pool, mm_psum, ew, carry_pool)
            if loop_iters == 1:
                _emit_body(tc_, y, hT16d, hT8d, w16_sb, w8_sb, pools)
            else:
                with tc_.For_i(0, loop_iters, 1):
                    _emit_body(tc_, y, hT16d, hT8d, w16_sb, w8_sb, pools)
    nc.compile()
    return nc


_CACHED_NC = None


def _prep_inputs(h, W0, W1):
    W = np.stack([np.asarray(W0), np.asarray(W1)]).astype(np.float32)
    w16 = np.ascontiguousarray(W[:, K8:, :] * WS).astype(np.float16)
    base = {"w16": w16}
    if NK8:
        base["w8"] = np.ascontiguousarray(W[:, :K8, :] * WS).astype(NP8)
    maps = []
    for c in range(8):
        hT = np.ascontiguousarray(np.asarray(h[c]).T)   # [H, T] f32
        m = dict(base)
        m["hT16"] = hT.astype(np.float16)
        if NK8:
            m["hT8"] = np.ascontiguousarray(hT[:K8]).astype(NP8)
        maps.append(m)
    return maps


def kernel(h, W0, W1):
    global _CACHED_NC
    if _CACHED_NC is None:
        _CACHED_NC = build_nc()
    res = bass_utils.run_bass_kernel_spmd(
        _CACHED_NC, _prep_inputs(h, W0, W1), core_ids=list(range(8)))
    return np.stack([res.results[c]["y"].astype(np.float32).T for c in range(8)],
                    axis=0)


# revision 8
# speedup vs baseline: 2.2089x; 2.2089x over previous
"""MinGRU (2-layer) Trainium2 Bass kernel.

Problem: h[8,4096,1024] f32, W0/W1 [1024,3072] f32.
Per layer: z = h @ W; hidden,gate,proj = split(z);
  a = sigmoid(-gate); g_hidden = relu(hidden) + min(sigmoid(hidden), 0.5)
  scan: out_t = a_t*out_{t-1} + (1-a_t)*g_hidden_t   (DVE scan, fp32 state)
  h = sigmoid(proj)*out + (1-sigmoid(proj))*h

Sharding: batch row per core (B=8 over 8 cores), weights replicated.
On-core layout: [feat(partitions), time] throughout; host pre-transposes the
inputs and post-transposes the [H,T] fp16 output during unshard, so the PE
does zero transposes.

Matmul precision: contraction K=1024 is split — the first NK8*256 rows run
as fp8e4 DoubleRow matmuls (2 k-tiles per MM, half the PE issue slots), the
rest as fp16 matmuls, all accumulating into the same fp32 PSUM. Weights are
pre-scaled by WS=16 on host (keeps fp8 out of subnormals); ACT un-scales via
its activation scale input. Measured end-to-end rel L2 error ~1.3e-2 at
NK8=2 (vs the 2e-2 gate; fp16-only is ~5.5e-4).

Engine balance per [128,512] block (cost model): ACT 3 sigmoids (612ns ea),
Pool relu+2 fused stt (427ns ea), DVE scan (594ns) + 3 fp16 tensor_tensor
(327ns ea). All under the PE matmul stream, which is the bottleneck.
"""

import os
import sys

if "/opt/trn_rl_repo" not in sys.path:
    sys.path.insert(0, "/opt/trn_rl_repo")

from contextlib import ExitStack

import numpy as np

import concourse.bass as bass
import concourse.tile as tile
from concourse import bacc, mybir
from concourse import bass_utils

T, H, H3 = 4096, 1024, 3072
TC = 512                 # time chunk (= one PSUM bank of fp32)
NCHUNK = T // TC
NFB = H // 128           # feature blocks (partition tiles) = k-tiles total
NK8 = int(os.environ.get("MINGRU_NK8", "2"))  # fp8 DoubleRow k-tile PAIRS
POOL_EW = int(os.environ.get("MINGRU_POOL", "0"))  # GPSIMD in elemwise chain
NK16 = NFB - 2 * NK8     # fp16 k-tiles
K8 = NK8 * 256           # contraction rows done in fp8
WS = 16.0                # weight pre-scale; PSUM holds WS*z, ACT divides out
F32 = mybir.dt.float32
F16 = mybir.dt.float16
F8 = mybir.dt.float8e4
NP8 = mybir.dt.np(F8)
ACT = mybir.ActivationFunctionType
ALU = mybir.AluOpType
DRMODE = mybir.MatmulPerfMode.DoubleRow


def _emit_elemwise(nc, ew, carries, rhs16, h1T16, h1T8, y, ph, pg, pp, i, li, f):
    s_ = ew.tile([128, TC], F16, tag="s")
    nc.scalar.activation(s_[:], ph[:], ACT.Sigmoid, scale=1.0 / WS)
    a_ = ew.tile([128, TC], F16, tag="a")
    nc.scalar.activation(a_[:], pg[:], ACT.Sigmoid, scale=-1.0 / WS)
    g_ = ew.tile([128, TC], F16, tag="g")
    nc.scalar.activation(g_[:], pp[:], ACT.Sigmoid, scale=1.0 / WS)
    # GPSIMD cannot touch PSUM and only runs tensor_tensor/tensor_scalar/
    # copy; POOL_EW toggles whether it carries part of the chain (its real
    # HW throughput is far below the cost model's, so default off).
    rh = ew.tile([128, TC], F16, tag="rh")
    if POOL_EW:
        nc.vector.tensor_scalar(rh[:], ph[:], 0.0, 1.0 / WS,
                                op0=ALU.max, op1=ALU.mult)
    else:
        nc.scalar.activation(rh[:], ph[:], ACT.Relu, scale=1.0 / WS)
    smin = ew.tile([128, TC], F16, tag="smin")
    nc.vector.tensor_scalar_min(smin[:], s_[:], 0.5)
    am1 = ew.tile([128, TC], F16, tag="am1")
    eng_am1 = nc.gpsimd if POOL_EW else nc.vector
    eng_am1.tensor_scalar_add(am1[:], a_[:], -1.0)
    gh = ew.tile([128, TC], F16, tag="gh")
    eng_gh = nc.gpsimd if POOL_EW else nc.vector
    eng_gh.tensor_tensor(gh[:], smin[:], rh[:], op=ALU.add)
    negb = ew.tile([128, TC], F16, tag="negb")
    eng_negb = nc.gpsimd if POOL_EW else nc.vector
    eng_negb.tensor_tensor(negb[:], am1[:], gh[:], op=ALU.mult)
    sc = ew.tile([128, TC], F16, tag="sc")
    col = li * NFB + f
    init = 0.0 if i == 0 else carries[:, col:col + 1]
    nc.vector.tensor_tensor_scan(sc[:], a_[:], negb[:], init,
                                 op0=ALU.mult, op1=ALU.subtract)
    if i < NCHUNK - 1:
        nc.vector.tensor_copy(carries[:, col:col + 1], sc[:, TC - 1:TC])
    # highway: h_out = h_in + g*(sc - h_in)
    d = ew.tile([128, TC], F16, tag="d")
    nc.vector.tensor_sub(d[:], sc[:], rhs16[:, f, :])
    m = ew.tile([128, TC], F16, tag="m")
    nc.vector.tensor_mul(m[:], g_[:], d[:])
    eng_out = nc.gpsimd if POOL_EW else nc.vector
    if li == 0:
        eng_out.tensor_add(h1T16[:, f, :], m[:], rhs16[:, f, :])
        if f < 2 * NK8:
            eng_out.tensor_copy(h1T8[:, f, :], h1T16[:, f, :])
    else:
        yt = ew.tile([128, TC], F16, tag="yt", bufs=3)
        eng_out.tensor_add(yt[:], m[:], rhs16[:, f, :])
        nc.sync.dma_start(y[f * 128:(f + 1) * 128, i * TC:(i + 1) * TC], yt[:])


def _emit_body(tc_, y, hT16d, hT8d, w16_sb, w8_sb, pools):
    nc = tc_.nc
    hT_pool, mm_psum, ew, carry_pool = pools
    carries = carry_pool.tile([128, 2 * NFB], F16)

    def emit_layer(i, li, rhs16, rhs8, h1T16, h1T8):
        for f in range(NFB):
            ph = mm_psum.tile([128, TC], F32, tag="ph")
            pg = mm_psum.tile([128, TC], F32, tag="pg")
            pp = mm_psum.tile([128, TC], F32, tag="pp")
            nstep = NK8 + NK16
            step = 0
            for kp in range(NK8):
                st = dict(start=(step == 0), stop=(step == nstep - 1))
                lw8 = w8_sb[li][kp]
                r8 = rhs8[:, 2 * kp:2 * kp + 2, :]
                nc.tensor.matmul(ph[:], lw8[:, :, f * 128:(f + 1) * 128],
                                 r8, perf_mode=DRMODE, **st)
                nc.tensor.matmul(pg[:], lw8[:, :, H + f * 128:H + (f + 1) * 128],
                                 r8, perf_mode=DRMODE, **st)
                nc.tensor.matmul(pp[:], lw8[:, :, 2 * H + f * 128:2 * H + (f + 1) * 128],
                                 r8, perf_mode=DRMODE, **st)
                step += 1
            for j in range(NK16):
                st = dict(start=(step == 0), stop=(step == nstep - 1))
                lw = w16_sb[li][j]
                r = rhs16[:, 2 * NK8 + j, :]
                nc.tensor.matmul(ph[:], lw[:, f * 128:(f + 1) * 128], r, **st)
                nc.tensor.matmul(pg[:], lw[:, H + f * 128:H + (f + 1) * 128], r, **st)
                nc.tensor.matmul(pp[:], lw[:, 2 * H + f * 128:2 * H + (f + 1) * 128],
                                 r, **st)
                step += 1
            _emit_elemwise(nc, ew, carries, rhs16, h1T16, h1T8, y,
                           ph, pg, pp, i, li, f)

    # Layer-2 runs one chunk behind layer-1 so the PE never stalls on the
    # DVE/Pool chain that produces layer-2's rhs.
    prev = None
    for i in range(NCHUNK):
        hT16 = hT_pool.tile([128, NFB, TC], F16, tag="hT")
        for k in range(NFB):
            nc.sync.dma_start(hT16[:, k, :],
                              hT16d[k * 128:(k + 1) * 128, i * TC:(i + 1) * TC])
        hT8 = None
        if NK8:
            hT8 = hT_pool.tile([128, 2 * NK8, TC], F8, tag="hT8")
            for k in range(2 * NK8):
                nc.sync.dma_start(hT8[:, k, :],
                                  hT8d[k * 128:(k + 1) * 128, i * TC:(i + 1) * TC])
        h1T16 = hT_pool.tile([128, NFB, TC], F16, tag="h1T")
        h1T8 = None
        if NK8:
            h1T8 = hT_pool.tile([128, 2 * NK8, TC], F8, tag="h1T8", name="h1T8")
        emit_layer(i, 0, hT16, hT8, h1T16, h1T8)
        if prev is not None:
            emit_layer(i - 1, 1, prev[0], prev[1], None, None)
        prev = (h1T16, h1T8)
    emit_layer(NCHUNK - 1, 1, prev[0], prev[1], None, None)


def build_nc(loop_iters: int = 1):
    """Build + compile the per-core Bass program (SPMD across 8 cores)."""
    nc = bacc.Bacc("TRN2", target_bir_lowering=False, debug=False,
                   enable_asserts=False, num_devices=8)
    hT16d = nc.dram_tensor("hT16", [H, T], F16, kind="ExternalInput").ap()
    hT8d = (nc.dram_tensor("hT8", [K8, T], F8, kind="ExternalInput").ap()
            if NK8 else None)
    w16d = nc.dram_tensor("w16", [2, H - K8, H3], F16, kind="ExternalInput").ap()
    w8d = (nc.dram_tensor("w8", [2, K8, H3], F8, kind="ExternalInput").ap()
           if NK8 else None)
    y = nc.dram_tensor("y", [H, T], F16, kind="ExternalOutput").ap()

    with tile.TileContext(nc) as tc_:
        with ExitStack() as ctx:
            wpool = ctx.enter_context(tc_.tile_pool(name="w", bufs=1))
            const = ctx.enter_context(tc_.tile_pool(name="const", bufs=1))
            hT_pool = ctx.enter_context(tc_.tile_pool(name="hT", bufs=2))
            mm_psum = ctx.enter_context(
                tc_.tile_pool(name="mmp", bufs=2, space="PSUM"))
            ew = ctx.enter_context(tc_.tile_pool(name="ew", bufs=2))
            carry_pool = ctx.enter_context(tc_.tile_pool(name="carry", bufs=1))

            # weights: layer-1 tiles first so chunk-0 matmuls gate on the
            # least possible DMA
            w16_sb = [[wpool.tile([128, H3], F16, name=f"w16_{li}_{j}",
                                  tag=f"w16_{li}_{j}")
                       for j in range(NK16)] for li in range(2)]
            w8_sb = [[wpool.tile([128, 2, H3], F8, name=f"w8_{li}_{kp}",
                                 tag=f"w8_{li}_{kp}")
                      for kp in range(NK8)] for li in range(2)]
            for li in range(2):
                for kp in range(NK8):
                    for jj in range(2):
                        nc.sync.dma_start(
                            w8_sb[li][kp][:, jj, :],
                            w8d[li, kp * 256 + jj * 128:kp * 256 + (jj + 1) * 128, :])
                for j in range(NK16):
                    nc.sync.dma_start(w16_sb[li][j][:],
                                      w16d[li, j * 128:(j + 1) * 128, :])

            # PE clock (HAM) warmup + ACT sigmoid-table preload, fed from a
            # memset tile so nothing gates on DMA.
            warm = const.tile([128, 128], F32, name="warm")
            nc.vector.memset(warm[:], 0.0)
            warm_ps = mm_psum.tile([128, TC], F32, tag="ph")
            for _ in range(16):
                nc.tensor.matmul(warm_ps[:, 0:128], warm[:], warm[:],
                                 start=True, stop=True)
            warm_sb = ew.tile([128, TC], F16, tag="s")
            nc.scalar.activation(warm_sb[:, 0:1], warm_ps[:, 0:1], ACT.Sigmoid)

            pools = (hT_pool, mm_psum, ew, carry_pool)
            if loop_iters == 1:
                _emit_body(tc_, y, hT16d, hT8d, w16_sb, w8_sb, pools)
            else:
                with tc_.For_i(0, loop_iters, 1):
                    _emit_body(tc_, y, hT16d, hT8d, w16_sb, w8_sb, pools)
    nc.compile()
    return nc


_CACHED_NC = None


def _prep_inputs(h, W0, W1):
    W = np.stack([np.asarray(W0), np.asarray(W1)]).astype(np.float32)
    w16 = np.ascontiguousarray(W[:, K8:, :] * WS).astype(np.float16)
    base = {"w16": w16}
    if NK8:
        base["w8"] = np.ascontiguousarray(W[:, :K8, :] * WS).astype(NP8)
    maps = []
    for c in range(8):
        hT = np.ascontiguousarray(np.asarray(h[c]).T)   # [H, T] f32
        m = dict(base)
        m["hT16"] = hT.astype(np.float16)
        if NK8:
            m["hT8"] = np.ascontiguousarray(hT[:K8]).astype(NP8)
        maps.append(m)
    return maps


def kernel(h, W0, W1):
    global _CACHED_NC
    if _CACHED_NC is None:
        _CACHED_NC = build_nc()
    res = bass_utils.run_bass_kernel_spmd(
        _CACHED_NC, _prep_inputs(h, W0, W1), core_ids=list(range(8)))
    return np.stack([res.results[c]["y"].astype(np.float32).T for c in range(8)],
                    axis=0)


# revision 12
# speedup vs baseline: 3.2971x; 1.4927x over previous
"""MinGRU (2-layer) Trainium2 Bass kernel.

Problem: h[8,4096,1024] f32, W0/W1 [1024,3072] f32.
Per layer: z = h @ W; hidden,gate,proj = split(z);
  a = sigmoid(-gate); g_hidden = relu(hidden) + min(sigmoid(hidden), 0.5)
  scan: out_t = a_t*out_{t-1} + (1-a_t)*g_hidden_t   (DVE scan, fp32 state)
  h = sigmoid(proj)*out + (1-sigmoid(proj))*h

Sharding: batch row per core (B=8 over 8 cores), weights replicated.
On-core layout: [feat(partitions), time] throughout; host pre-transposes the
inputs and post-transposes the [H,T] fp16 output during unshard, so the PE
does zero transposes.

Matmul precision: contraction K=1024 is split — the first NK8*256 rows run
as fp8e4 DoubleRow matmuls (2 k-tiles per MM, half the PE issue slots), the
rest as fp16 matmuls, all accumulating into the same fp32 PSUM. Weights are
pre-scaled by WS=16 on host (keeps fp8 out of subnormals); ACT un-scales via
its activation scale input. Measured end-to-end rel L2 error ~1.3e-2 at
NK8=2 (vs the 2e-2 gate; fp16-only is ~5.5e-4).

Engine balance per [128,512] block (cost model): ACT 3 sigmoids (612ns ea),
Pool relu+2 fused stt (427ns ea), DVE scan (594ns) + 3 fp16 tensor_tensor
(327ns ea). All under the PE matmul stream, which is the bottleneck.
"""

import os
import sys

if "/opt/trn_rl_repo" not in sys.path:
    sys.path.insert(0, "/opt/trn_rl_repo")

from contextlib import ExitStack

import numpy as np

import concourse.bass as bass
import concourse.tile as tile
from concourse import bacc, mybir
from concourse import bass_utils

T, H, H3 = 4096, 1024, 3072
TC = 512                 # time chunk (= one PSUM bank of fp32)
NCHUNK = T // TC
NFB = H // 128           # feature blocks (partition tiles) = k-tiles total
NK8 = int(os.environ.get("MINGRU_NK8", "2"))  # fp8 DoubleRow k-tile PAIRS
POOL_EW = int(os.environ.get("MINGRU_POOL", "0"))  # GPSIMD in elemwise chain
NK16 = NFB - 2 * NK8     # fp16 k-tiles
K8 = NK8 * 256           # contraction rows done in fp8
WS = 16.0                # weight pre-scale; PSUM holds WS*z, ACT divides out
F32 = mybir.dt.float32
F16 = mybir.dt.float16
F8 = mybir.dt.float8e4
NP8 = mybir.dt.np(F8)
ACT = mybir.ActivationFunctionType
ALU = mybir.AluOpType
DRMODE = mybir.MatmulPerfMode.DoubleRow


def _emit_elemwise(nc, ew, carries, rhs16, h1T16, h1T8, y, ph, pg, pp, i, li, f):
    # ph's two readers (s, rh) go first so its PSUM bank frees two ACT ops
    # earlier, unblocking the next f-group's matmuls.
    s_ = ew.tile([128, TC], F16, tag="s")
    nc.scalar.activation(s_[:], ph[:], ACT.Sigmoid, scale=1.0 / WS)
    # GPSIMD cannot touch PSUM and only runs tensor_tensor/tensor_scalar/
    # copy; POOL_EW toggles whether it carries part of the chain (its real
    # HW throughput is far below the cost model's, so default off).
    rh = ew.tile([128, TC], F16, tag="rh")
    if POOL_EW:
        nc.vector.tensor_scalar(rh[:], ph[:], 0.0, 1.0 / WS,
                                op0=ALU.max, op1=ALU.mult)
    else:
        nc.scalar.activation(rh[:], ph[:], ACT.Relu, scale=1.0 / WS)
    a_ = ew.tile([128, TC], F16, tag="a")
    nc.scalar.activation(a_[:], pg[:], ACT.Sigmoid, scale=-1.0 / WS)
    g_ = ew.tile([128, TC], F16, tag="g")
    nc.scalar.activation(g_[:], pp[:], ACT.Sigmoid, scale=1.0 / WS)
    smin = ew.tile([128, TC], F16, tag="smin")
    nc.vector.tensor_scalar_min(smin[:], s_[:], 0.5)
    am1 = ew.tile([128, TC], F16, tag="am1")
    eng_am1 = nc.gpsimd if POOL_EW else nc.vector
    eng_am1.tensor_scalar_add(am1[:], a_[:], -1.0)
    gh = ew.tile([128, TC], F16, tag="gh")
    eng_gh = nc.gpsimd if POOL_EW else nc.vector
    eng_gh.tensor_tensor(gh[:], smin[:], rh[:], op=ALU.add)
    negb = ew.tile([128, TC], F16, tag="negb")
    eng_negb = nc.gpsimd if POOL_EW else nc.vector
    eng_negb.tensor_tensor(negb[:], am1[:], gh[:], op=ALU.mult)
    sc = ew.tile([128, TC], F16, tag="sc")
    col = li * NFB + f
    init = 0.0 if i == 0 else carries[:, col:col + 1]
    nc.vector.tensor_tensor_scan(sc[:], a_[:], negb[:], init,
                                 op0=ALU.mult, op1=ALU.subtract)
    if i < NCHUNK - 1:
        nc.vector.tensor_copy(carries[:, col:col + 1], sc[:, TC - 1:TC])
    # highway: h_out = h_in + g*(sc - h_in)
    d = ew.tile([128, TC], F16, tag="d")
    nc.vector.tensor_sub(d[:], sc[:], rhs16[:, f, :])
    m = ew.tile([128, TC], F16, tag="m")
    nc.vector.tensor_mul(m[:], g_[:], d[:])
    eng_out = nc.gpsimd if POOL_EW else nc.vector
    if li == 0:
        eng_out.tensor_add(h1T16[:, f, :], m[:], rhs16[:, f, :])
        if f < 2 * NK8:
            eng_out.tensor_copy(h1T8[:, f, :], h1T16[:, f, :])
    else:
        yt = ew.tile([128, TC], F16, tag="yt", bufs=3)
        eng_out.tensor_add(yt[:], m[:], rhs16[:, f, :])
        nc.sync.dma_start(y[f * 128:(f + 1) * 128, i * TC:(i + 1) * TC], yt[:])


def _emit_body(tc_, y, hT16d, hT8d, w16_sb, w8_sb, pools):
    nc = tc_.nc
    hT_pool, mm_psum, ew, carry_pool = pools
    carries = carry_pool.tile([128, 2 * NFB], F16)

    def emit_layer(i, li, rhs16, rhs8, h1T16, h1T8):
        for f in range(NFB):
            ph = mm_psum.tile([128, TC], F32, tag="ph")
            pg = mm_psum.tile([128, TC], F32, tag="pg")
            pp = mm_psum.tile([128, TC], F32, tag="pp")
            # grouped k-step order (DR first) measured faster than
            # interleaving on HW
            steps = ([("dr", kp) for kp in range(NK8)] +
                     [("f16", j) for j in range(NK16)])
            nstep = len(steps)
            for step, (kind, idx) in enumerate(steps):
                st = dict(start=(step == 0), stop=(step == nstep - 1))
                if kind == "dr":
                    lw8 = w8_sb[li][idx]
                    r8 = rhs8[:, 2 * idx:2 * idx + 2, :]
                    nc.tensor.matmul(ph[:], lw8[:, :, f * 128:(f + 1) * 128],
                                     r8, perf_mode=DRMODE, **st)
                    nc.tensor.matmul(pg[:], lw8[:, :, H + f * 128:H + (f + 1) * 128],
                                     r8, perf_mode=DRMODE, **st)
                    nc.tensor.matmul(pp[:],
                                     lw8[:, :, 2 * H + f * 128:2 * H + (f + 1) * 128],
                                     r8, perf_mode=DRMODE, **st)
                else:
                    lw = w16_sb[li][idx]
                    r = rhs16[:, 2 * NK8 + idx, :]
                    nc.tensor.matmul(ph[:], lw[:, f * 128:(f + 1) * 128], r, **st)
                    nc.tensor.matmul(pg[:], lw[:, H + f * 128:H + (f + 1) * 128],
                                     r, **st)
                    nc.tensor.matmul(pp[:], lw[:, 2 * H + f * 128:2 * H + (f + 1) * 128],
                                     r, **st)
            _emit_elemwise(nc, ew, carries, rhs16, h1T16, h1T8, y,
                           ph, pg, pp, i, li, f)

    # Layer-2 runs one chunk behind layer-1 so the PE never stalls on the
    # DVE/Pool chain that produces layer-2's rhs.
    prev = None
    for i in range(NCHUNK):
        hT16 = hT_pool.tile([128, NFB, TC], F16, tag="hT")
        for k in range(NFB):
            nc.sync.dma_start(hT16[:, k, :],
                              hT16d[k * 128:(k + 1) * 128, i * TC:(i + 1) * TC])
        hT8 = None
        if NK8:
            hT8 = hT_pool.tile([128, 2 * NK8, TC], F8, tag="hT8")
            for k in range(2 * NK8):
                nc.sync.dma_start(hT8[:, k, :],
                                  hT8d[k * 128:(k + 1) * 128, i * TC:(i + 1) * TC])
        h1T16 = hT_pool.tile([128, NFB, TC], F16, tag="h1T")
        h1T8 = None
        if NK8:
            h1T8 = hT_pool.tile([128, 2 * NK8, TC], F8, tag="h1T8", name="h1T8")
        emit_layer(i, 0, hT16, hT8, h1T16, h1T8)
        if prev is not None:
            emit_layer(i - 1, 1, prev[0], prev[1], None, None)
        prev = (h1T16, h1T8)
    emit_layer(NCHUNK - 1, 1, prev[0], prev[1], None, None)


def build_nc(loop_iters: int = 1):
    """Build + compile the per-core Bass program (SPMD across 8 cores)."""
    nc = bacc.Bacc("TRN2", target_bir_lowering=False, debug=False,
                   enable_asserts=False, num_devices=8)
    hT16d = nc.dram_tensor("hT16", [H, T], F16, kind="ExternalInput").ap()
    hT8d = (nc.dram_tensor("hT8", [K8, T], F8, kind="ExternalInput").ap()
            if NK8 else None)
    w16d = nc.dram_tensor("w16", [2, H - K8, H3], F16, kind="ExternalInput").ap()
    w8d = (nc.dram_tensor("w8", [2, K8, H3], F8, kind="ExternalInput").ap()
           if NK8 else None)
    y = nc.dram_tensor("y", [H, T], F16, kind="ExternalOutput").ap()

    with tile.TileContext(nc) as tc_:
        with ExitStack() as ctx:
            wpool = ctx.enter_context(tc_.tile_pool(name="w", bufs=1))
            const = ctx.enter_context(tc_.tile_pool(name="const", bufs=1))
            hT_pool = ctx.enter_context(tc_.tile_pool(name="hT", bufs=2))
            mm_psum = ctx.enter_context(
                tc_.tile_pool(name="mmp", bufs=2, space="PSUM"))
            ew = ctx.enter_context(tc_.tile_pool(name="ew", bufs=3))
            carry_pool = ctx.enter_context(tc_.tile_pool(name="carry", bufs=1))

            # weights: layer-1 tiles first so chunk-0 matmuls gate on the
            # least possible DMA
            w16_sb = [[wpool.tile([128, H3], F16, name=f"w16_{li}_{j}",
                                  tag=f"w16_{li}_{j}")
                       for j in range(NK16)] for li in range(2)]
            w8_sb = [[wpool.tile([128, 2, H3], F8, name=f"w8_{li}_{kp}",
                                 tag=f"w8_{li}_{kp}")
                      for kp in range(NK8)] for li in range(2)]
            for li in range(2):
                for kp in range(NK8):
                    for jj in range(2):
                        nc.sync.dma_start(
                            w8_sb[li][kp][:, jj, :],
                            w8d[li, kp * 256 + jj * 128:kp * 256 + (jj + 1) * 128, :])
                for j in range(NK16):
                    nc.sync.dma_start(w16_sb[li][j][:],
                                      w16d[li, j * 128:(j + 1) * 128, :])

            # PE clock (HAM) warmup + ACT sigmoid-table preload, fed from a
            # memset tile so nothing gates on DMA.
            warm = const.tile([128, 128], F32, name="warm")
            nc.vector.memset(warm[:], 0.0)
            warm_ps = mm_psum.tile([128, TC], F32, tag="ph")
            for _ in range(16):
                nc.tensor.matmul(warm_ps[:, 0:128], warm[:], warm[:],
                                 start=True, stop=True)
            warm_sb = ew.tile([128, TC], F16, tag="s")
            nc.scalar.activation(warm_sb[:, 0:1], warm_ps[:, 0:1], ACT.Sigmoid)

            pools = (hT_pool, mm_psum, ew, carry_pool)
            if loop_iters == 1:
                _emit_body(tc_, y, hT16d, hT8d, w16_sb, w8_sb, pools)
            else:
                with tc_.For_i(0, loop_iters, 1):
                    _emit_body(tc_, y, hT16d, hT8d, w16_sb, w8_sb, pools)
    nc.compile()
    return nc


_CACHED_NC = None


def _prep_inputs(h, W0, W1):
    W = np.stack([np.asarray(W0), np.asarray(W1)]).astype(np.float32)
    w16 = np.ascontiguousarray(W[:, K8:, :] * WS).astype(np.float16)
    base = {"w16": w16}
    if NK8:
        base["w8"] = np.ascontiguousarray(W[:, :K8, :] * WS).astype(NP8)
    maps = []
    for c in range(8):
        hT = np.ascontiguousarray(np.asarray(h[c]).T)   # [H, T] f32
        m = dict(base)
        m["hT16"] = hT.astype(np.float16)
        if NK8:
            m["hT8"] = np.ascontiguousarray(hT[:K8]).astype(NP8)
        maps.append(m)
    return maps


def kernel(h, W0, W1):
    global _CACHED_NC
    if _CACHED_NC is None:
        _CACHED_NC = build_nc()
    res = bass_utils.run_bass_kernel_spmd(
        _CACHED_NC, _prep_inputs(h, W0, W1), core_ids=list(range(8)))
    return np.stack([res.results[c]["y"].astype(np.float32).T for c in range(8)],
                    axis=0)
